# revision 10
# baseline (speedup 1.0000x reference)
"""ALFE block (patch-merge LN + spatial/channel attention + 1x1 conv +
bilinear upsample + residual) as a distributed Bass kernel on 8 TRN2
NeuronCores.

Sharding: core = (batch b, vertical half).  Each core receives the full
(H-rolled) image of its batch, computes patch-merge/LN/qkv over all 4096
half-res pixels, spatial attention only for its own 34-row query window
(32 output rows + 1 halo row on each side, so the bilinear upsample seam
needs no cross-core traffic), and writes a [64, 64, 128] output slab.
The per-core roll makes the query window a fixed [0:2176) column range on
every core, so all 8 cores run one SPMD program; per-core differences
live in the data (rolled input, boundary-clamp masks).
"""

import sys

sys.path.insert(0, "/opt/trn_rl_repo")

import contextlib
import ctypes
import types

import numpy as np
import ml_dtypes

import concourse.bass as bass
import concourse.tile as tile
from concourse import mybir
from concourse.masks import make_identity

# ---------------------------------------------------------------- infra shims
# 1) walrus in this container rejects InstDrain with >2 sync waits; re-emit
#    the tile exit drain's waits as standalone SP wait_ge instructions.


def _patched_drain_and_barrier(self, tick_clock, wait_clock):
    from concourse.vector_clock import ScopedClock

    nc = self.nc
    dummy = mybir.InstNoOp(name="I-drain-wait-probe", ins=[], outs=[])
    dummy.engine = mybir.EngineType.SP
    wait_clock.add_sem_waits(dummy, ScopedClock({None: tick_clock.global_clock}))
    si = dummy.sync_info
    assert self.sems is not None
    id2h = {h.num: h for h in self.sems.allocated().values()}
    if si is not None:
        for w in si.on_wait:
            assert w.wait_mode == "sem-ge-imm", w
            nc.sync.wait_ge(id2h[w.id], w.wait_value)
    nc.sync.drain()
    nc.all_engine_barrier()
    popped = nc._tile_sem_poison_stack.pop()
    assert popped is self._sem_poison
    nc.clear_and_free_semaphores(list(self.sems.allocated().values()))
    nc.all_engine_barrier()


tile.TileContext._drain_and_barrier = _patched_drain_and_barrier


def _split_excess_waits(nc, limit=1):
    """walrus here rejects instructions with more than ~2 sync waits; hoist
    excess waits onto standalone InstEventSemaphore instructions inserted
    just before the over-subscribed instruction on the same engine."""
    n_split = 0
    for f in nc.m.functions:
        for b in f.blocks:
            insts = list(b.instructions)
            out = []
            for inst in insts:
                si = inst.sync_info
                waits = list(si.on_wait) if si is not None else []
                if len(waits) > limit:
                    keep = waits[: limit - 1] if limit > 1 else []
                    hoist = waits[limit - 1 :] if limit > 1 else waits
                    # leave room: keep limit-1 on the instruction, then one
                    # hoisted event-sem per remaining wait
                    for w in hoist[:-1] if limit > 1 else hoist:
                        ev = mybir.InstEventSemaphore(
                            name=f"I-waitsplit-{nc.next_id()}", ins=[], outs=[]
                        )
                        ev.engine = inst.engine
                        ev.sync_info = mybir.SyncInfo(on_wait=[w], on_update=[])
                        nc.register_instruction(ev)
                        out.append(ev)
                        n_split += 1
                    if limit > 1:
                        keep = keep + [hoist[-1]]
                    inst.sync_info = mybir.SyncInfo(
                        on_wait=keep, on_update=list(si.on_update)
                    )
                out.append(inst)
            b.instructions = out
    return n_split

# 2) antenv.axon_hooks is missing in this image; provide it so
#    run_bass_kernel_spmd(trace=True) can capture NTFF profiles.


def _install_ntff_hook():
    def _make_hook():
        try:
            lib = ctypes.CDLL("/opt/axon/libaxon_pjrt.so")
        except OSError:
            return None
        if not hasattr(lib, "axon_start_nrt_profile"):
            return None
        lib.axon_start_nrt_profile.argtypes = [
            ctypes.POINTER(ctypes.c_int64),
            ctypes.c_size_t,
        ]
        lib.axon_start_nrt_profile.restype = ctypes.c_int64
        lib.axon_stop_nrt_profile.argtypes = [ctypes.c_char_p]
        lib.axon_stop_nrt_profile.restype = ctypes.c_int64

        @contextlib.contextmanager
        def _hook(output_dir, device_ids):
            import jax

            jax.devices()
            if device_ids:
                ids = (ctypes.c_int64 * len(device_ids))(*device_ids)
                rc = lib.axon_start_nrt_profile(ids, len(device_ids))
            else:
                rc = lib.axon_start_nrt_profile(None, 0)
            if rc != 0:
                raise RuntimeError(f"axon_start_nrt_profile rc={rc}")
            try:
                yield
            finally:
                n = lib.axon_stop_nrt_profile(str(output_dir).encode())
                print(f"ntff profile: {n} file(s) -> {output_dir}", file=sys.stderr)

        return _hook

    hook = _make_hook()
    mod = types.ModuleType("antenv.axon_hooks")
    mod.get_axon_ntff_profile_hook = lambda: hook
    mod.set_axon_ntff_profile_hook = lambda h: None
    sys.modules.setdefault("antenv.axon_hooks", mod)


_install_ntff_hook()

# ------------------------------------------------------------------ constants
B, C, H, W = 4, 64, 128, 128
Hh, Wh = H // 2, W // 2          # 64, 64
NQ = Hh * Wh                     # 4096 half-res pixels
C4 = 4 * C                       # 256
WIN = 34                         # query-window rows (32 out + 1 halo each side)
JW = WIN * Wh                    # 2176 query columns
JPASSES = [(0, 1024), (1024, 1024), (2048, 128)]

F32 = mybir.dt.float32
BF16 = mybir.dt.bfloat16
BF = ml_dtypes.bfloat16

EXP = mybir.ActivationFunctionType.Exp
SQRT = mybir.ActivationFunctionType.Sqrt
MULT = mybir.AluOpType.mult
ADD = mybir.AluOpType.add
SUB = mybir.AluOpType.subtract


# ------------------------------------------------------------- device program
def build_program():
    nc = bass.Bass("TRN2", target_bir_lowering=False, debug=False, num_devices=8)

    xm_e = nc.dram_tensor("xm", [128, 2, NQ], F32, kind="ExternalInput").ap()
    xres_e = nc.dram_tensor("xres", [64, Hh, W], F32, kind="ExternalInput").ap()
    lnw_e = nc.dram_tensor("lnw", [128, 4, 66], BF16, kind="ExternalInput").ap()
    wq_e = nc.dram_tensor("wq", [64, 64], BF16, kind="ExternalInput").ap()
    wk_e = nc.dram_tensor("wk", [64, 64], BF16, kind="ExternalInput").ap()
    wv_e = nc.dram_tensor("wv", [64, 64], BF16, kind="ExternalInput").ap()
    wpix_e = nc.dram_tensor("wpix", [64, 192], BF16, kind="ExternalInput").ap()
    w1t_e = nc.dram_tensor("w1t", [128, 64], BF16, kind="ExternalInput").ap()
    bvec_e = nc.dram_tensor("bvec", [64, 10], F32, kind="ExternalInput").ap()
    brow_e = nc.dram_tensor("brow", [1, 192], F32, kind="ExternalInput").ap()
    out_e = nc.dram_tensor("out", [64, Hh, W], F32, kind="ExternalOutput").ap()

    with tile.TileContext(nc) as tc:
        with (
            tc.tile_pool(name="consts", bufs=1) as consts,
            tc.tile_pool(name="persist", bufs=1) as big,
            tc.tile_pool(name="norm", bufs=2) as normp,
            tc.tile_pool(name="dscr", bufs=1, space="DRAM") as dscr,
        ):
            # ---- constant loads
            lnw = consts.tile([128, 4, 66], BF16)
            nc.sync.dma_start(out=lnw, in_=lnw_e)
            wq = consts.tile([64, 64], BF16)
            nc.sync.dma_start(out=wq, in_=wq_e)
            wk = consts.tile([64, 64], BF16)
            nc.sync.dma_start(out=wk, in_=wk_e)
            wv = consts.tile([64, 64], BF16)
            nc.sync.dma_start(out=wv, in_=wv_e)
            wpix = consts.tile([64, 192], BF16)
            nc.sync.dma_start(out=wpix, in_=wpix_e)
            w1t = consts.tile([128, 64], BF16)
            nc.sync.dma_start(out=w1t, in_=w1t_e)
            bvec = consts.tile([64, 10], F32)
            nc.sync.dma_start(out=bvec, in_=bvec_e)
            brow = consts.tile([128, 192], F32)
            nc.sync.dma_start(out=brow, in_=brow_e[0:1, :].to_broadcast((128, 192)))
            eps = consts.tile([128, 1], F32)
            nc.gpsimd.memset(eps, 1e-5)
            ident = consts.tile([64, 64], BF16)
            make_identity(nc, ident)

            # ---- input + elementwise prep
            xres = big.tile([64, Hh, W], F32)
            nc.sync.dma_start(out=xres, in_=xres_e)
            c1 = big.tile([64, NQ], BF16)

            # ---- phase 1: patch-merge linear with folded LayerNorm
            with (
                tc.tile_pool(name="ph1buf", bufs=1) as ph1b,
                tc.tile_pool(name="ps1", bufs=1, space="PSUM") as ps1,
            ):
                xm = ph1b.tile([128, 2, NQ], F32)
                nc.sync.dma_start(out=xm, in_=xm_e)
                mbf = ph1b.tile([128, 2, NQ], BF16)
                nc.gpsimd.tensor_copy(out=mbf, in_=xm)
                m2bf = ph1b.tile([128, 2, NQ], BF16)
                nc.vector.tensor_mul(out=m2bf, in0=xm, in1=xm)
                ps_ln = ps1.tile([66, NQ], F32)
                for jt in range(8):
                    sl = slice(jt * 512, (jt + 1) * 512)
                    for ck in range(4):
                        rhs = (mbf if ck < 2 else m2bf)[:, ck % 2, sl]
                        nc.tensor.matmul(
                            ps_ln[:, sl],
                            lhsT=lnw[:, ck, :],
                            rhs=rhs,
                            start=(ck == 0),
                            stop=(ck == 3),
                        )
                # stats rows -> SBUF -> DRAM -> [128, 2, 32] working layout
                stat_sb = ph1b.tile([66, NQ], BF16, tag="statsb")
                nc.vector.tensor_copy(out=stat_sb[64:66, :], in_=ps_ln[64:66, :])
                dstat = dscr.tile([2, NQ], BF16)
                nc.sync.dma_start(out=dstat, in_=stat_sb[64:66, :])
                st = consts.tile([128, 2, 32], BF16)
                nc.sync.dma_start(
                    out=st, in_=dstat[:, :].rearrange("k (p t) -> p k t", t=32)
                )
                mu = consts.tile([128, 32], F32)
                nc.vector.tensor_scalar_mul(out=mu, in0=st[:, 0, :], scalar1=1.0 / C4)
                var = consts.tile([128, 32], F32)
                nc.vector.tensor_mul(out=var, in0=mu, in1=mu)
                nc.vector.scalar_tensor_tensor(
                    out=var, in0=st[:, 1, :], scalar=1.0 / C4, in1=var,
                    op0=MULT, op1=SUB,
                )
                sa = consts.tile([128, 32], F32)  # sqrt(var+eps) = 1/rstd
                nc.scalar.activation(out=sa, in_=var, func=SQRT, bias=eps)
                ra = consts.tile([128, 32], F32)  # rstd
                nc.vector.reciprocal(out=ra, in_=sa)
                negmu_bf = consts.tile([128, 32], BF16)
                nc.vector.tensor_scalar_mul(out=negmu_bf, in0=mu, scalar1=-1.0)
                sa_bf = consts.tile([128, 32], BF16)
                nc.vector.tensor_copy(out=sa_bf, in_=sa)
                ra_bf = consts.tile([128, 32], BF16)
                nc.vector.tensor_copy(out=ra_bf, in_=ra)

                drow = dscr.tile([3, NQ], BF16)
                for r, src in ((0, negmu_bf), (1, sa_bf), (2, ra_bf)):
                    nc.sync.dma_start(
                        out=drow[r].rearrange("(p t) -> p t", t=32), in_=src
                    )
                a_bc = ph1b.tile([64, NQ], BF16)
                nc.sync.dma_start(
                    out=a_bc, in_=drow[2:3, :].to_broadcast((64, NQ))
                )

                # c1 = (pre + G*(-mu) + Bc*(1/rstd)) * rstd, per 512-slice
                for jt in range(8):
                    sl = slice(jt * 512, (jt + 1) * 512)
                    nb = normp.tile([64, 512], BF16, tag="nb")
                    nc.sync.dma_start(
                        out=nb, in_=drow[0:1, sl].to_broadcast((64, 512))
                    )
                    sb_ = normp.tile([64, 512], BF16, tag="sb2")
                    nc.sync.dma_start(
                        out=sb_, in_=drow[1:2, sl].to_broadcast((64, 512))
                    )
                    t1 = normp.tile([64, 512], F32, tag="t1")
                    nc.vector.scalar_tensor_tensor(
                        out=t1, in0=nb, scalar=bvec[:, 8:9],
                        in1=ps_ln[0:64, sl], op0=MULT, op1=ADD,
                    )
                    nc.vector.scalar_tensor_tensor(
                        out=t1, in0=sb_, scalar=bvec[:, 9:10],
                        in1=t1, op0=MULT, op1=ADD,
                    )
                    nc.vector.tensor_mul(
                        out=c1[:, sl], in0=t1, in1=a_bc[:, sl]
                    )

            # ---- phase 2: q/k/v, pixel-major combo, channel attention
            q = big.tile([64, JW], BF16)
            k = big.tile([64, NQ], BF16)
            v = big.tile([64, JW], BF16)
            pix = big.tile([128, 32, 3, 65], BF16)  # [vT|1], q2T, k2T per chunk
            nc.gpsimd.memset(pix[:, :, 0, 64:65], 1.0)
            cat = big.tile([128, JW], BF16)

            with tc.tile_pool(name="ps2", bufs=2, space="PSUM") as ps2:
                for jt in range(8):
                    sl = slice(jt * 512, (jt + 1) * 512)
                    ps_k = ps2.tile([64, 512], F32, tag="mm512")
                    nc.tensor.matmul(
                        ps_k, lhsT=wk, rhs=c1[:, sl], start=True, stop=True
                    )
                    nc.vector.tensor_scalar_add(
                        out=k[:, sl], in0=ps_k, scalar1=bvec[:, 1:2]
                    )
                for jt in range(5):
                    j0 = jt * 512
                    jw = min(512, JW - j0)
                    sl = slice(j0, j0 + jw)
                    ps_q = ps2.tile([64, 512], F32, tag="mm512")
                    nc.tensor.matmul(
                        ps_q[:, 0:jw], lhsT=wq, rhs=c1[:, sl], start=True, stop=True
                    )
                    nc.vector.tensor_scalar_add(
                        out=q[:, sl], in0=ps_q[:, 0:jw], scalar1=bvec[:, 0:1]
                    )
                    ps_v = ps2.tile([64, 512], F32, tag="mm512")
                    nc.tensor.matmul(
                        ps_v[:, 0:jw], lhsT=wv, rhs=c1[:, sl], start=True, stop=True
                    )
                    nc.vector.tensor_scalar_add(
                        out=v[:, sl], in0=ps_v[:, 0:jw], scalar1=bvec[:, 2:3]
                    )
                # pixel-major [vT | q2T | k2T] (+b4,b5,b6) in one pass
                for it in range(32):
                    sl = slice(it * 128, (it + 1) * 128)
                    ps_px = ps2.tile([128, 192], F32, tag="px")
                    nc.tensor.matmul(
                        ps_px, lhsT=c1[:, sl], rhs=wpix, start=True, stop=True
                    )
                    nc.vector.tensor_add(
                        out=pix[:, it, :, 0:64],
                        in0=ps_px[:, :].rearrange("p (a b) -> p a b", b=64),
                        in1=brow[:, :].rearrange("p (a b) -> p a b", b=64),
                    )
                # channel attention
                ps_s2 = ps2.tile([64, 64], F32, tag="tiny")
                for it in range(32):
                    nc.tensor.matmul(
                        ps_s2,
                        lhsT=pix[:, it, 1, 0:64],
                        rhs=pix[:, it, 2, 0:64],
                        start=(it == 0),
                        stop=(it == 31),
                    )
                e2 = consts.tile([64, 64], F32)
                rs2 = consts.tile([64, 1], F32)
                nc.scalar.activation(out=e2, in_=ps_s2, func=EXP, accum_out=rs2)
                rr2 = consts.tile([64, 1], F32)
                nc.vector.reciprocal(out=rr2, in_=rs2)
                p2 = consts.tile([64, 64], BF16)
                nc.vector.tensor_scalar_mul(out=p2, in0=e2, scalar1=rr2)
                ps_t = ps2.tile([64, 64], BF16, tag="tinyT")
                nc.tensor.transpose(ps_t, in_=p2, identity=ident)
                p2t = consts.tile([64, 64], BF16)
                nc.vector.tensor_copy(out=p2t, in_=ps_t)
                out2 = big.tile([64, JW], BF16)
                for jt in range(5):
                    j0 = jt * 512
                    jw = min(512, JW - j0)
                    ps_o2 = ps2.tile([64, 512], F32, tag="mm512")
                    nc.tensor.matmul(
                        ps_o2[:, 0:jw], lhsT=p2t, rhs=v[:, j0 : j0 + jw],
                        start=True, stop=True,
                    )
                    nc.vector.tensor_copy(
                        out=out2[:, j0 : j0 + jw], in_=ps_o2[:, 0:jw]
                    )
                nc.sync.dma_start(out=cat[64:128, :], in_=out2)

            # ---- phase 3: spatial attention (transposed scores, deferred
            #      softmax normalization via ones-row denominators) + conv1
            z = big.tile([64, WIN, Wh], BF16)
            z2 = z[:, :, :].rearrange("c h w -> c (h w)")
            dz = dscr.tile([1, JW], F32)
            with (
                tc.tile_pool(name="ps3", bufs=2, space="PSUM") as ps3,
                tc.tile_pool(name="psacc", bufs=1, space="PSUM") as psacc,
                tc.tile_pool(name="etp", bufs=3) as etp,
                tc.tile_pool(name="ph3n", bufs=2) as ph3n,
            ):
                for (j0, jw) in JPASSES:
                    ps_acc = psacc.tile([65, 1024], F32, tag="acc")
                    for it in range(32):
                        isl = slice(it * 128, (it + 1) * 128)
                        ps_s = ps3.tile([128, 1024], F32, tag="s")
                        for h in range(0, jw, 512):
                            hw = min(512, jw - h)
                            nc.tensor.matmul(
                                ps_s[:, h : h + hw],
                                lhsT=k[:, isl],
                                rhs=q[:, j0 + h : j0 + h + hw],
                                start=True,
                                stop=True,
                            )
                        eT = etp.tile([128, 1024], BF16, tag="eT")
                        nc.scalar.activation(
                            out=eT[:, 0:jw], in_=ps_s[:, 0:jw], func=EXP
                        )
                        for h in range(0, jw, 512):
                            hw = min(512, jw - h)
                            nc.tensor.matmul(
                                ps_acc[:, h : h + hw],
                                lhsT=pix[:, it, 0, :],
                                rhs=eT[:, h : h + hw],
                                start=(it == 0),
                                stop=(it == 31),
                            )
                    # normalize out1 by the ones-row denominator
                    rd = ph3n.tile([65, 1024], F32, tag="rd")
                    nc.vector.reciprocal(
                        out=rd[64:65, 0:jw], in_=ps_acc[64:65, 0:jw]
                    )
                    nc.sync.dma_start(
                        out=dz[0:1, j0 : j0 + jw], in_=rd[64:65, 0:jw]
                    )
                    rb = ph3n.tile([64, 1024], F32, tag="rb")
                    nc.sync.dma_start(
                        out=rb[:, 0:jw],
                        in_=dz[0:1, j0 : j0 + jw].to_broadcast((64, jw)),
                    )
                    nc.vector.tensor_mul(
                        out=cat[0:64, j0 : j0 + jw],
                        in0=ps_acc[0:64, 0:jw],
                        in1=rb[:, 0:jw],
                    )
                    # conv1 for this j-range
                    for h in range(0, jw, 512):
                        hw = min(512, jw - h)
                        ps_z = ps3.tile([64, 512], F32, tag="z")
                        nc.tensor.matmul(
                            ps_z[:, 0:hw],
                            lhsT=w1t,
                            rhs=cat[:, j0 + h : j0 + h + hw],
                            start=True,
                            stop=True,
                        )
                        nc.vector.tensor_scalar_add(
                            out=z2[:, j0 + h : j0 + h + hw],
                            in0=ps_z[:, 0:hw],
                            scalar1=bvec[:, 5:6],
                        )

            # ---- phase 4: bilinear upsample x2 + residual
            tailb = tc.alloc_tile_pool(name="tailbuf", bufs=1)
            dv = tailb.tile([64, 33, Wh], BF16)  # z'[t] - z'[t+1]
            nc.vector.tensor_sub(out=dv, in0=z[:, 0:33, :], in1=z[:, 1:34, :])
            upv = tailb.tile([64, Hh, Wh], BF16)
            upv_r = upv[:, :, :].rearrange("c (t two) w -> c t two w", two=2)
            nc.vector.scalar_tensor_tensor(
                out=upv_r[:, :, 0, :], in0=dv[:, 0:32, :], scalar=0.25,
                in1=z[:, 1:33, :], op0=MULT, op1=ADD,
            )
            nc.vector.scalar_tensor_tensor(
                out=upv_r[:, :, 1, :], in0=dv[:, 1:33, :], scalar=-0.25,
                in1=z[:, 1:33, :], op0=MULT, op1=ADD,
            )
            # boundary clamp corrections (maskA/maskB nonzero on edge cores)
            nc.vector.scalar_tensor_tensor(
                out=upv[:, 0, :], in0=dv[:, 0, :], scalar=bvec[:, 6:7],
                in1=upv[:, 0, :], op0=MULT, op1=ADD,
            )
            nc.vector.scalar_tensor_tensor(
                out=upv[:, Hh - 1, :], in0=dv[:, 32, :], scalar=bvec[:, 7:8],
                in1=upv[:, Hh - 1, :], op0=MULT, op1=ADD,
            )
            # horizontal
            dhh = tailb.tile([64, Hh, 63], BF16)
            nc.vector.tensor_sub(
                out=dhh, in0=upv[:, :, 0:63], in1=upv[:, :, 1:64]
            )
            uph = tailb.tile([64, Hh, W], BF16)
            uph_r = uph[:, :, :].rearrange("c h (s two) -> c h s two", two=2)
            nc.vector.scalar_tensor_tensor(
                out=uph_r[:, :, 1:64, 0], in0=dhh, scalar=0.25,
                in1=upv[:, :, 1:64], op0=MULT, op1=ADD,
            )
            nc.vector.scalar_tensor_tensor(
                out=uph_r[:, :, 0:63, 1], in0=dhh, scalar=-0.25,
                in1=upv[:, :, 0:63], op0=MULT, op1=ADD,
            )
            nc.vector.tensor_copy(out=uph_r[:, :, 0, 0], in_=upv[:, :, 0])
            nc.vector.tensor_copy(out=uph_r[:, :, 63, 1], in_=upv[:, :, 63])
            # residual add, f32 out
            outb = tailb.tile([64, Hh, W], F32)
            nc.vector.tensor_add(out=outb, in0=uph, in1=xres)
            nc.sync.dma_start(out=out_e, in_=outb)
            tailb.release()

    _split_excess_waits(nc)
    return nc


# ------------------------------------------------------------- host-side prep
def prepare_params(
    pm_gamma, pm_beta, pm_w, pm_b, w1, b1, w2, b2, w3, b3, w4, b4, w5, b5, w6, b6
):
    f = np.float32
    pm_gamma, pm_beta, pm_w, pm_b = (
        np.asarray(a, f) for a in (pm_gamma, pm_beta, pm_w, pm_b)
    )
    wg = pm_w * pm_gamma[None, :]           # [64, 256]
    G = wg.sum(1)                           # [64]
    Bc = pm_w @ pm_beta + pm_b              # [64]
    lnw = np.zeros((128, 4, 66), f)
    for ck in range(2):
        lnw[:, ck, 0:64] = wg[:, ck * 128 : (ck + 1) * 128].T
        lnw[:, ck, 64] = 1.0
    lnw[:, 2, 65] = 1.0
    lnw[:, 3, 65] = 1.0
    wpix = np.concatenate(
        [np.asarray(w4, f).T, np.asarray(w5, f).T, np.asarray(w6, f).T], axis=1
    )
    brow = np.concatenate(
        [np.asarray(b4, f), np.asarray(b5, f), np.asarray(b6, f)]
    ).reshape(1, 192)
    common = {
        "lnw": np.ascontiguousarray(lnw.astype(BF)),
        "wq": np.ascontiguousarray(np.asarray(w2, f).T.astype(BF)),
        "wk": np.ascontiguousarray(np.asarray(w3, f).T.astype(BF)),
        "wv": np.ascontiguousarray(np.asarray(w4, f).T.astype(BF)),
        "wpix": np.ascontiguousarray(wpix.astype(BF)),
        "w1t": np.ascontiguousarray(np.asarray(w1, f).T.astype(BF)),
        "brow": brow,
    }
    bv = np.zeros((64, 10), f)
    for i, b in enumerate((b2, b3, b4, b5, b6, b1)):
        bv[:, i] = np.asarray(b, f)
    bv[:, 8] = G
    bv[:, 9] = Bc
    return common, bv


def make_xm(xb):
    """rolled x[b] [64, 128, 128] -> quadrant layout [128, 2, 4096]."""
    m = np.concatenate(
        [xb[:, 0::2, 0::2], xb[:, 1::2, 0::2], xb[:, 0::2, 1::2], xb[:, 1::2, 1::2]],
        axis=0,
    ).reshape(C4, NQ)
    return np.ascontiguousarray(m.reshape(2, 128, NQ).transpose(1, 0, 2))


def make_in_maps(inputs):
    x = np.asarray(inputs["x"], np.float32)
    common, bv = prepare_params(**{kk: vv for kk, vv in inputs.items() if kk != "x"})
    in_maps = []
    for core in range(8):
        b, half = core // 2, core % 2
        shift = 2 - 64 * half  # rolled[rf] = real[rf - shift]
        xr = np.roll(x[b], shift, axis=1)
        bvc = bv.copy()
        bvc[:, 6] = -0.25 if half == 0 else 0.0
        bvc[:, 7] = 0.25 if half == 1 else 0.0
        xres = np.ascontiguousarray(x[b][:, 64 * half : 64 * half + 64, :])
        in_maps.append(dict(common, xm=make_xm(xr), xres=xres, bvec=bvc))
    return in_maps


def gather(results):
    out = np.zeros((B, C, H, W), np.float32)
    for core in range(8):
        b, half = core // 2, core % 2
        out[b, :, 64 * half : 64 * half + 64, :] = results[core]["out"]
    return out


_NC = None


def _get_nc():
    global _NC
    if _NC is None:
        _NC = build_program()
    return _NC


def run(inputs, trace=False, tmpdir=None):
    from concourse.bass_utils import run_bass_kernel_spmd

    res = run_bass_kernel_spmd(
        _get_nc(),
        make_in_maps(inputs),
        core_ids=list(range(8)),
        trace=trace,
        tmpdir=tmpdir,
    )
    return gather(res.results), res


def kernel(**inputs):
    return run(inputs)[0]


# revision 14
# speedup vs baseline: 1.1915x; 1.1915x over previous
"""ALFE block (patch-merge LN + spatial/channel attention + 1x1 conv +
bilinear upsample + residual) as a distributed Bass kernel on 8 TRN2
NeuronCores.

Sharding: core = (batch b, vertical half).  Each core receives the full
(H-rolled) image of its batch, computes patch-merge/LN/qkv over all 4096
half-res pixels, spatial attention only for its own 34-row query window
(32 output rows + 1 halo row on each side, so the bilinear upsample seam
needs no cross-core traffic), and writes a [64, 64, 128] output slab.
The per-core roll makes the query window a fixed [0:2176) column range on
every core, so all 8 cores run one SPMD program; per-core differences
live in the data (rolled input, boundary-clamp masks).
"""

import sys

sys.path.insert(0, "/opt/trn_rl_repo")

import contextlib
import ctypes
import types

import numpy as np
import ml_dtypes

import concourse.bass as bass
import concourse.tile as tile
from concourse import mybir
from concourse.masks import make_identity

# ---------------------------------------------------------------- infra shims
# 1) walrus in this container rejects InstDrain with >2 sync waits; re-emit
#    the tile exit drain's waits as standalone SP wait_ge instructions.


def _patched_drain_and_barrier(self, tick_clock, wait_clock):
    from concourse.vector_clock import ScopedClock

    nc = self.nc
    dummy = mybir.InstNoOp(name="I-drain-wait-probe", ins=[], outs=[])
    dummy.engine = mybir.EngineType.SP
    wait_clock.add_sem_waits(dummy, ScopedClock({None: tick_clock.global_clock}))
    si = dummy.sync_info
    assert self.sems is not None
    id2h = {h.num: h for h in self.sems.allocated().values()}
    if si is not None:
        for w in si.on_wait:
            assert w.wait_mode == "sem-ge-imm", w
            nc.sync.wait_ge(id2h[w.id], w.wait_value)
    nc.sync.drain()
    nc.all_engine_barrier()
    popped = nc._tile_sem_poison_stack.pop()
    assert popped is self._sem_poison
    nc.clear_and_free_semaphores(list(self.sems.allocated().values()))
    nc.all_engine_barrier()


tile.TileContext._drain_and_barrier = _patched_drain_and_barrier


def _split_excess_waits(nc, limit=1):
    """walrus here rejects instructions with more than ~2 sync waits; hoist
    excess waits onto standalone InstEventSemaphore instructions inserted
    just before the over-subscribed instruction on the same engine."""
    n_split = 0
    for f in nc.m.functions:
        for b in f.blocks:
            insts = list(b.instructions)
            out = []
            for inst in insts:
                si = inst.sync_info
                waits = list(si.on_wait) if si is not None else []
                if len(waits) > limit:
                    keep = waits[: limit - 1] if limit > 1 else []
                    hoist = waits[limit - 1 :] if limit > 1 else waits
                    # leave room: keep limit-1 on the instruction, then one
                    # hoisted event-sem per remaining wait
                    for w in hoist[:-1] if limit > 1 else hoist:
                        ev = mybir.InstEventSemaphore(
                            name=f"I-waitsplit-{nc.next_id()}", ins=[], outs=[]
                        )
                        ev.engine = inst.engine
                        ev.sync_info = mybir.SyncInfo(on_wait=[w], on_update=[])
                        nc.register_instruction(ev)
                        out.append(ev)
                        n_split += 1
                    if limit > 1:
                        keep = keep + [hoist[-1]]
                    inst.sync_info = mybir.SyncInfo(
                        on_wait=keep, on_update=list(si.on_update)
                    )
                out.append(inst)
            b.instructions = out
    return n_split

# 2) antenv.axon_hooks is missing in this image; provide it so
#    run_bass_kernel_spmd(trace=True) can capture NTFF profiles.


def _install_ntff_hook():
    def _make_hook():
        try:
            lib = ctypes.CDLL("/opt/axon/libaxon_pjrt.so")
        except OSError:
            return None
        if not hasattr(lib, "axon_start_nrt_profile"):
            return None
        lib.axon_start_nrt_profile.argtypes = [
            ctypes.POINTER(ctypes.c_int64),
            ctypes.c_size_t,
        ]
        lib.axon_start_nrt_profile.restype = ctypes.c_int64
        lib.axon_stop_nrt_profile.argtypes = [ctypes.c_char_p]
        lib.axon_stop_nrt_profile.restype = ctypes.c_int64

        @contextlib.contextmanager
        def _hook(output_dir, device_ids):
            import jax

            jax.devices()
            if device_ids:
                ids = (ctypes.c_int64 * len(device_ids))(*device_ids)
                rc = lib.axon_start_nrt_profile(ids, len(device_ids))
            else:
                rc = lib.axon_start_nrt_profile(None, 0)
            if rc != 0:
                raise RuntimeError(f"axon_start_nrt_profile rc={rc}")
            try:
                yield
            finally:
                n = lib.axon_stop_nrt_profile(str(output_dir).encode())
                print(f"ntff profile: {n} file(s) -> {output_dir}", file=sys.stderr)

        return _hook

    hook = _make_hook()
    mod = types.ModuleType("antenv.axon_hooks")
    mod.get_axon_ntff_profile_hook = lambda: hook
    mod.set_axon_ntff_profile_hook = lambda h: None
    sys.modules.setdefault("antenv.axon_hooks", mod)


_install_ntff_hook()

# ------------------------------------------------------------------ constants
B, C, H, W = 4, 64, 128, 128
Hh, Wh = H // 2, W // 2          # 64, 64
NQ = Hh * Wh                     # 4096 half-res pixels
C4 = 4 * C                       # 256
WIN = 34                         # query-window rows (32 out + 1 halo each side)
JW = WIN * Wh                    # 2176 query columns
JPASSES = [(0, 1024), (1024, 1024), (2048, 128)]

F32 = mybir.dt.float32
BF16 = mybir.dt.bfloat16
BF = ml_dtypes.bfloat16

EXP = mybir.ActivationFunctionType.Exp
SQRT = mybir.ActivationFunctionType.Sqrt
MULT = mybir.AluOpType.mult
ADD = mybir.AluOpType.add
SUB = mybir.AluOpType.subtract


# ------------------------------------------------------------- device program
def build_program():
    nc = bass.Bass("TRN2", target_bir_lowering=False, debug=False, num_devices=8)

    xmb_e = nc.dram_tensor("xmb", [128, 2, NQ], BF16, kind="ExternalInput").ap()
    xm2b_e = nc.dram_tensor("xm2b", [128, 2, NQ], BF16, kind="ExternalInput").ap()
    xres_e = nc.dram_tensor("xres", [64, Hh, W], F32, kind="ExternalInput").ap()
    lnw_e = nc.dram_tensor("lnw", [128, 2, 64], BF16, kind="ExternalInput").ap()
    lnst_e = nc.dram_tensor("lnst", [128, 4, 2], BF16, kind="ExternalInput").ap()
    gbt_e = nc.dram_tensor("gbt", [2, 64], BF16, kind="ExternalInput").ap()
    wq_e = nc.dram_tensor("wq", [64, 64], BF16, kind="ExternalInput").ap()
    wk_e = nc.dram_tensor("wk", [64, 64], BF16, kind="ExternalInput").ap()
    wv_e = nc.dram_tensor("wv", [64, 64], BF16, kind="ExternalInput").ap()
    wpix_e = nc.dram_tensor("wpix", [64, 192], BF16, kind="ExternalInput").ap()
    w1t_e = nc.dram_tensor("w1t", [128, 64], BF16, kind="ExternalInput").ap()
    bvec_e = nc.dram_tensor("bvec", [64, 10], F32, kind="ExternalInput").ap()
    brow_e = nc.dram_tensor("brow", [1, 192], F32, kind="ExternalInput").ap()
    out_e = nc.dram_tensor("out", [64, Hh, W], F32, kind="ExternalOutput").ap()

    with tile.TileContext(nc) as tc:
        with (
            tc.tile_pool(name="consts", bufs=1) as consts,
            tc.tile_pool(name="persist", bufs=1) as big,
            tc.tile_pool(name="norm", bufs=2) as normp,
            tc.tile_pool(name="dscr", bufs=1, space="DRAM") as dscr,
        ):
            # ---- constant loads
            lnw = consts.tile([128, 2, 64], BF16)
            nc.sync.dma_start(out=lnw, in_=lnw_e)
            lnst = consts.tile([128, 4, 2], BF16)
            nc.sync.dma_start(out=lnst, in_=lnst_e)
            gbt = consts.tile([2, 64], BF16)
            nc.sync.dma_start(out=gbt, in_=gbt_e)
            wq = consts.tile([64, 64], BF16)
            nc.sync.dma_start(out=wq, in_=wq_e)
            wk = consts.tile([64, 64], BF16)
            nc.sync.dma_start(out=wk, in_=wk_e)
            wv = consts.tile([64, 64], BF16)
            nc.sync.dma_start(out=wv, in_=wv_e)
            wpix = consts.tile([64, 192], BF16)
            nc.sync.dma_start(out=wpix, in_=wpix_e)
            w1t = consts.tile([128, 64], BF16)
            nc.sync.dma_start(out=w1t, in_=w1t_e)
            bvec = consts.tile([64, 10], F32)
            nc.sync.dma_start(out=bvec, in_=bvec_e)
            brow = consts.tile([128, 192], F32)
            nc.sync.dma_start(out=brow, in_=brow_e[0:1, :].to_broadcast((128, 192)))
            eps = consts.tile([128, 1], F32)
            nc.gpsimd.memset(eps, 1e-5)
            ident = consts.tile([64, 64], BF16)
            make_identity(nc, ident)

            # PE warm-up: ~6us of dummy matmuls so HAM unthrottles while the
            # input DMA is in flight
            wdum = consts.tile([128, 512], BF16)
            nc.gpsimd.memset(wdum, 0.25)
            with tc.tile_pool(name="psw", bufs=1, space="PSUM") as psw:
                ps_w = psw.tile([128, 512], F32)
                for _ in range(14):
                    nc.tensor.matmul(
                        ps_w, lhsT=wdum[:, 0:128], rhs=wdum,
                        start=True, stop=True,
                    )

            # ---- input + elementwise prep
            xres = big.tile([64, Hh, W], F32)
            nc.sync.dma_start(out=xres, in_=xres_e)
            c1 = big.tile([64, NQ], BF16)

            # ---- phase 1: patch-merge linear with folded LayerNorm.
            # Stats (col-sums of m, m^2) go to small psum tiles first; the
            # main linear runs afterwards with a K=2 fixup row pair carrying
            # the G*(-mu) and Bc*(1/rstd) terms, so no wide psum barrier.
            ps1 = tc.alloc_tile_pool(name="ps12", bufs=2, space="PSUM")
            ps2 = ps1  # shared psum pool across phases 1-2 (no bank barrier)
            ph1b = tc.alloc_tile_pool(name="ph1buf", bufs=1)
            if True:
                mbf = ph1b.tile([128, 2, NQ], BF16)
                nc.sync.dma_start(out=mbf, in_=xmb_e)
                m2bf = ph1b.tile([128, 2, NQ], BF16)
                nc.sync.dma_start(out=m2bf, in_=xm2b_e)
                stat_sb = ph1b.tile([2, NQ], BF16, tag="statsb")
                for jt in range(8):
                    sl = slice(jt * 512, (jt + 1) * 512)
                    ps_st = ps1.tile([2, 512], F32, tag="st")
                    for ck in range(4):
                        rhs = (mbf if ck < 2 else m2bf)[:, ck % 2, sl]
                        nc.tensor.matmul(
                            ps_st,
                            lhsT=lnst[:, ck, :],
                            rhs=rhs,
                            start=(ck == 0),
                            stop=(ck == 3),
                        )
                    nc.vector.tensor_copy(out=stat_sb[:, sl], in_=ps_st)
                dstat = dscr.tile([2, NQ], BF16)
                nc.sync.dma_start(out=dstat, in_=stat_sb)
                st = consts.tile([128, 2, 32], BF16)
                nc.sync.dma_start(
                    out=st, in_=dstat[:, :].rearrange("k (p t) -> p k t", t=32)
                )
                mu = consts.tile([128, 32], F32)
                nc.vector.tensor_scalar_mul(out=mu, in0=st[:, 0, :], scalar1=1.0 / C4)
                var = consts.tile([128, 32], F32)
                nc.vector.tensor_mul(out=var, in0=mu, in1=mu)
                nc.vector.scalar_tensor_tensor(
                    out=var, in0=st[:, 1, :], scalar=1.0 / C4, in1=var,
                    op0=MULT, op1=SUB,
                )
                sa = consts.tile([128, 32], F32)  # sqrt(var+eps) = 1/rstd
                nc.scalar.activation(out=sa, in_=var, func=SQRT, bias=eps)
                ra = consts.tile([128, 32], F32)  # rstd
                nc.vector.reciprocal(out=ra, in_=sa)
                negmu_bf = consts.tile([128, 32], BF16)
                nc.vector.tensor_scalar_mul(out=negmu_bf, in0=mu, scalar1=-1.0)
                sa_bf = consts.tile([128, 32], BF16)
                nc.vector.tensor_copy(out=sa_bf, in_=sa)
                ra_bf = consts.tile([128, 32], BF16)
                nc.vector.tensor_copy(out=ra_bf, in_=ra)

                drow = dscr.tile([3, NQ], BF16)
                for r, src in ((0, negmu_bf), (1, sa_bf), (2, ra_bf)):
                    nc.sync.dma_start(
                        out=drow[r].rearrange("(p t) -> p t", t=32), in_=src
                    )
                a_bc = ph1b.tile([64, NQ], BF16)
                nc.sync.dma_start(
                    out=a_bc, in_=drow[2:3, :].to_broadcast((64, NQ))
                )
                nsrow = ph1b.tile([2, NQ], BF16)
                nc.sync.dma_start(out=nsrow, in_=drow[0:2, :])

                # main linear + K=2 stats fixup, then scale by rstd
                for jt in range(8):
                    sl = slice(jt * 512, (jt + 1) * 512)
                    ps_c = ps1.tile([64, 512], F32, tag="c")
                    nc.tensor.matmul(
                        ps_c, lhsT=lnw[:, 0, :], rhs=mbf[:, 0, sl],
                        start=True, stop=False,
                    )
                    nc.tensor.matmul(
                        ps_c, lhsT=lnw[:, 1, :], rhs=mbf[:, 1, sl],
                        start=False, stop=False,
                    )
                    nc.tensor.matmul(
                        ps_c, lhsT=gbt, rhs=nsrow[:, sl],
                        start=False, stop=True,
                    )
                    nc.vector.tensor_mul(
                        out=c1[:, sl], in0=ps_c, in1=a_bc[:, sl]
                    )

            ph1b.release()

            # ---- phase 2: q/k/v, pixel-major combo, channel attention
            q = big.tile([64, JW], BF16)
            k = big.tile([64, NQ], BF16)
            v = big.tile([64, JW], BF16)
            pix = big.tile([128, 32, 3, 65], BF16)  # [vT|1], q2T, k2T per chunk
            nc.gpsimd.memset(pix[:, :, 0, 64:65], 1.0)
            cat = big.tile([128, JW], BF16)

            if True:
                for jt in range(8):
                    sl = slice(jt * 512, (jt + 1) * 512)
                    ps_k = ps2.tile([64, 512], F32, tag="c")
                    nc.tensor.matmul(
                        ps_k, lhsT=wk, rhs=c1[:, sl], start=True, stop=True
                    )
                    nc.vector.tensor_scalar_add(
                        out=k[:, sl], in0=ps_k, scalar1=bvec[:, 1:2]
                    )
                for jt in range(5):
                    j0 = jt * 512
                    jw = min(512, JW - j0)
                    sl = slice(j0, j0 + jw)
                    ps_q = ps2.tile([64, 512], F32, tag="c")
                    nc.tensor.matmul(
                        ps_q[:, 0:jw], lhsT=wq, rhs=c1[:, sl], start=True, stop=True
                    )
                    nc.vector.tensor_scalar_add(
                        out=q[:, sl], in0=ps_q[:, 0:jw], scalar1=bvec[:, 0:1]
                    )
                    ps_v = ps2.tile([64, 512], F32, tag="c")
                    nc.tensor.matmul(
                        ps_v[:, 0:jw], lhsT=wv, rhs=c1[:, sl], start=True, stop=True
                    )
                    nc.vector.tensor_scalar_add(
                        out=v[:, sl], in0=ps_v[:, 0:jw], scalar1=bvec[:, 2:3]
                    )
                # pixel-major [vT | q2T | k2T] (+b4,b5,b6) in one pass
                for it in range(32):
                    sl = slice(it * 128, (it + 1) * 128)
                    ps_px = ps2.tile([128, 192], F32, tag="px")
                    nc.tensor.matmul(
                        ps_px, lhsT=c1[:, sl], rhs=wpix, start=True, stop=True
                    )
                    nc.vector.tensor_add(
                        out=pix[:, it, :, 0:64],
                        in0=ps_px[:, :].rearrange("p (a b) -> p a b", b=64),
                        in1=brow[:, :].rearrange("p (a b) -> p a b", b=64),
                    )
                # channel attention
                ps_s2 = ps2.tile([64, 64], F32, tag="tiny")
                for it in range(32):
                    nc.tensor.matmul(
                        ps_s2,
                        lhsT=pix[:, it, 1, 0:64],
                        rhs=pix[:, it, 2, 0:64],
                        start=(it == 0),
                        stop=(it == 31),
                    )
                e2 = consts.tile([64, 64], F32)
                rs2 = consts.tile([64, 1], F32)
                nc.scalar.activation(out=e2, in_=ps_s2, func=EXP, accum_out=rs2)
                rr2 = consts.tile([64, 1], F32)
                nc.vector.reciprocal(out=rr2, in_=rs2)
                p2 = consts.tile([64, 64], BF16)
                nc.vector.tensor_scalar_mul(out=p2, in0=e2, scalar1=rr2)
                ps_t = ps2.tile([64, 64], BF16, tag="tiny")
                nc.tensor.transpose(ps_t, in_=p2, identity=ident)
                p2t = consts.tile([64, 64], BF16)
                nc.vector.tensor_copy(out=p2t, in_=ps_t)
                out2 = big.tile([64, JW], BF16)
                for jt in range(5):
                    j0 = jt * 512
                    jw = min(512, JW - j0)
                    ps_o2 = ps2.tile([64, 512], F32, tag="c")
                    nc.tensor.matmul(
                        ps_o2[:, 0:jw], lhsT=p2t, rhs=v[:, j0 : j0 + jw],
                        start=True, stop=True,
                    )
                    nc.vector.tensor_copy(
                        out=out2[:, j0 : j0 + jw], in_=ps_o2[:, 0:jw]
                    )
                nc.sync.dma_start(out=cat[64:128, :], in_=out2)
            ps1.release()

            # ---- phase 3: spatial attention (transposed scores, deferred
            #      softmax normalization via ones-row denominators) + conv1
            z = big.tile([64, WIN, Wh], BF16)
            z2 = z[:, :, :].rearrange("c h w -> c (h w)")
            dz = dscr.tile([1, JW], F32)
            with (
                tc.tile_pool(name="ps3", bufs=2, space="PSUM") as ps3,
                tc.tile_pool(name="psacc", bufs=2, space="PSUM") as psacc,
                tc.tile_pool(name="etp", bufs=3) as etp,
                tc.tile_pool(name="ph3n", bufs=2) as ph3n,
            ):
                for (j0, jw) in JPASSES:
                    ps_acc = psacc.tile([65, 1024], F32, tag="acc")
                    # group i-chunks so each exp op covers ~1024 psum columns
                    grp = max(1, 1024 // jw)
                    for g in range(0, 32, grp):
                        its = range(g, min(g + grp, 32))
                        ps_s = ps3.tile([128, 1024], F32, tag="s")
                        for c, it in enumerate(its):
                            isl = slice(it * 128, (it + 1) * 128)
                            for h in range(0, jw, 512):
                                hw = min(512, jw - h)
                                nc.tensor.matmul(
                                    ps_s[:, c * jw + h : c * jw + h + hw],
                                    lhsT=k[:, isl],
                                    rhs=q[:, j0 + h : j0 + h + hw],
                                    start=True,
                                    stop=True,
                                )
                        eT = etp.tile([128, 1024], BF16, tag="eT")
                        nw = len(its) * jw
                        nc.scalar.activation(
                            out=eT[:, 0:nw], in_=ps_s[:, 0:nw], func=EXP
                        )
                        for c, it in enumerate(its):
                            for h in range(0, jw, 512):
                                hw = min(512, jw - h)
                                nc.tensor.matmul(
                                    ps_acc[:, h : h + hw],
                                    lhsT=pix[:, it, 0, :],
                                    rhs=eT[:, c * jw + h : c * jw + h + hw],
                                    start=(it == 0),
                                    stop=(it == 31),
                                )
                    # normalize out1 by the ones-row denominator
                    rd = ph3n.tile([65, 1024], F32, tag="rd")
                    nc.vector.reciprocal(
                        out=rd[64:65, 0:jw], in_=ps_acc[64:65, 0:jw]
                    )
                    nc.sync.dma_start(
                        out=dz[0:1, j0 : j0 + jw], in_=rd[64:65, 0:jw]
                    )
                    rb = ph3n.tile([64, 1024], F32, tag="rb")
                    nc.sync.dma_start(
                        out=rb[:, 0:jw],
                        in_=dz[0:1, j0 : j0 + jw].to_broadcast((64, jw)),
                    )
                    nc.vector.tensor_mul(
                        out=cat[0:64, j0 : j0 + jw],
                        in0=ps_acc[0:64, 0:jw],
                        in1=rb[:, 0:jw],
                    )

            # conv1 over the whole window (after attention psum pools close)
            with tc.tile_pool(name="ps4", bufs=2, space="PSUM") as ps4:
                for h in range(0, JW, 512):
                    hw = min(512, JW - h)
                    ps_z = ps4.tile([64, 512], F32, tag="z")
                    nc.tensor.matmul(
                        ps_z[:, 0:hw],
                        lhsT=w1t,
                        rhs=cat[:, h : h + hw],
                        start=True,
                        stop=True,
                    )
                    nc.vector.tensor_scalar_add(
                        out=z2[:, h : h + hw],
                        in0=ps_z[:, 0:hw],
                        scalar1=bvec[:, 5:6],
                    )

            # ---- phase 4: bilinear upsample x2 + residual
            tailb = tc.alloc_tile_pool(name="tailbuf", bufs=1)
            dv = tailb.tile([64, 33, Wh], BF16)  # z'[t] - z'[t+1]
            nc.vector.tensor_sub(out=dv, in0=z[:, 0:33, :], in1=z[:, 1:34, :])
            upv = tailb.tile([64, Hh, Wh], BF16)
            upv_r = upv[:, :, :].rearrange("c (t two) w -> c t two w", two=2)
            nc.vector.scalar_tensor_tensor(
                out=upv_r[:, :, 0, :], in0=dv[:, 0:32, :], scalar=0.25,
                in1=z[:, 1:33, :], op0=MULT, op1=ADD,
            )
            nc.vector.scalar_tensor_tensor(
                out=upv_r[:, :, 1, :], in0=dv[:, 1:33, :], scalar=-0.25,
                in1=z[:, 1:33, :], op0=MULT, op1=ADD,
            )
            # boundary clamp corrections (maskA/maskB nonzero on edge cores)
            nc.vector.scalar_tensor_tensor(
                out=upv[:, 0, :], in0=dv[:, 0, :], scalar=bvec[:, 6:7],
                in1=upv[:, 0, :], op0=MULT, op1=ADD,
            )
            nc.vector.scalar_tensor_tensor(
                out=upv[:, Hh - 1, :], in0=dv[:, 32, :], scalar=bvec[:, 7:8],
                in1=upv[:, Hh - 1, :], op0=MULT, op1=ADD,
            )
            # horizontal
            dhh = tailb.tile([64, Hh, 63], BF16)
            nc.vector.tensor_sub(
                out=dhh, in0=upv[:, :, 0:63], in1=upv[:, :, 1:64]
            )
            uph = tailb.tile([64, Hh, W], BF16)
            uph_r = uph[:, :, :].rearrange("c h (s two) -> c h s two", two=2)
            nc.vector.scalar_tensor_tensor(
                out=uph_r[:, :, 1:64, 0], in0=dhh, scalar=0.25,
                in1=upv[:, :, 1:64], op0=MULT, op1=ADD,
            )
            nc.vector.scalar_tensor_tensor(
                out=uph_r[:, :, 0:63, 1], in0=dhh, scalar=-0.25,
                in1=upv[:, :, 0:63], op0=MULT, op1=ADD,
            )
            nc.vector.tensor_copy(out=uph_r[:, :, 0, 0], in_=upv[:, :, 0])
            nc.vector.tensor_copy(out=uph_r[:, :, 63, 1], in_=upv[:, :, 63])
            # residual add, f32 out
            outb = tailb.tile([64, Hh, W], F32)
            nc.vector.tensor_add(out=outb, in0=uph, in1=xres)
            nc.sync.dma_start(out=out_e, in_=outb)
            tailb.release()

    _split_excess_waits(nc)
    return nc


# ------------------------------------------------------------- host-side prep
def prepare_params(
    pm_gamma, pm_beta, pm_w, pm_b, w1, b1, w2, b2, w3, b3, w4, b4, w5, b5, w6, b6
):
    f = np.float32
    pm_gamma, pm_beta, pm_w, pm_b = (
        np.asarray(a, f) for a in (pm_gamma, pm_beta, pm_w, pm_b)
    )
    wg = pm_w * pm_gamma[None, :]           # [64, 256]
    G = wg.sum(1)                           # [64]
    Bc = pm_w @ pm_beta + pm_b              # [64]
    lnw = np.zeros((128, 2, 64), f)
    for ck in range(2):
        lnw[:, ck, :] = wg[:, ck * 128 : (ck + 1) * 128].T
    lnst = np.zeros((128, 4, 2), f)
    lnst[:, 0:2, 0] = 1.0
    lnst[:, 2:4, 1] = 1.0
    gbt = np.stack([G, Bc]).astype(f)        # [2, 64]
    wpix = np.concatenate(
        [np.asarray(w4, f).T, np.asarray(w5, f).T, np.asarray(w6, f).T], axis=1
    )
    brow = np.concatenate(
        [np.asarray(b4, f), np.asarray(b5, f), np.asarray(b6, f)]
    ).reshape(1, 192)
    common = {
        "lnw": np.ascontiguousarray(lnw.astype(BF)),
        "lnst": np.ascontiguousarray(lnst.astype(BF)),
        "gbt": np.ascontiguousarray(gbt.astype(BF)),
        "wq": np.ascontiguousarray(np.asarray(w2, f).T.astype(BF)),
        "wk": np.ascontiguousarray(np.asarray(w3, f).T.astype(BF)),
        "wv": np.ascontiguousarray(np.asarray(w4, f).T.astype(BF)),
        "wpix": np.ascontiguousarray(wpix.astype(BF)),
        "w1t": np.ascontiguousarray(np.asarray(w1, f).T.astype(BF)),
        "brow": brow,
    }
    bv = np.zeros((64, 10), f)
    for i, b in enumerate((b2, b3, b4, b5, b6, b1)):
        bv[:, i] = np.asarray(b, f)
    bv[:, 8] = G
    bv[:, 9] = Bc
    return common, bv


def make_xm(xb):
    """rolled x[b] [64, 128, 128] -> quadrant layout [128, 2, 4096]."""
    m = np.concatenate(
        [xb[:, 0::2, 0::2], xb[:, 1::2, 0::2], xb[:, 0::2, 1::2], xb[:, 1::2, 1::2]],
        axis=0,
    ).reshape(C4, NQ)
    return np.ascontiguousarray(m.reshape(2, 128, NQ).transpose(1, 0, 2))


def make_in_maps(inputs):
    x = np.asarray(inputs["x"], np.float32)
    common, bv = prepare_params(**{kk: vv for kk, vv in inputs.items() if kk != "x"})
    in_maps = []
    for core in range(8):
        b, half = core // 2, core % 2
        shift = 2 - 64 * half  # rolled[rf] = real[rf - shift]
        xr = np.roll(x[b], shift, axis=1)
        bvc = bv.copy()
        bvc[:, 6] = -0.25 if half == 0 else 0.0
        bvc[:, 7] = 0.25 if half == 1 else 0.0
        xres = np.ascontiguousarray(x[b][:, 64 * half : 64 * half + 64, :])
        m = make_xm(xr)
        in_maps.append(
            dict(
                common,
                xmb=np.ascontiguousarray(m.astype(BF)),
                xm2b=np.ascontiguousarray((m * m).astype(BF)),
                xres=xres,
                bvec=bvc,
            )
        )
    return in_maps


def gather(results):
    out = np.zeros((B, C, H, W), np.float32)
    for core in range(8):
        b, half = core // 2, core % 2
        out[b, :, 64 * half : 64 * half + 64, :] = results[core]["out"]
    return out


_NC = None


def _get_nc():
    global _NC
    if _NC is None:
        _NC = build_program()
    return _NC


def run(inputs, trace=False, tmpdir=None):
    from concourse.bass_utils import run_bass_kernel_spmd

    res = run_bass_kernel_spmd(
        _get_nc(),
        make_in_maps(inputs),
        core_ids=list(range(8)),
        trace=trace,
        tmpdir=tmpdir,
    )
    return gather(res.results), res


def kernel(**inputs):
    return run(inputs)[0]


# revision 16
# speedup vs baseline: 1.2879x; 1.0809x over previous
"""ALFE block (patch-merge LN + spatial/channel attention + 1x1 conv +
bilinear upsample + residual) as a distributed Bass kernel on 8 TRN2
NeuronCores.

Sharding: core = (batch b, vertical half).  Each core receives the full
(H-rolled) image of its batch, computes patch-merge/LN/qkv over all 4096
half-res pixels, spatial attention only for its own 34-row query window
(32 output rows + 1 halo row on each side, so the bilinear upsample seam
needs no cross-core traffic), and writes a [64, 64, 128] output slab.
The per-core roll makes the query window a fixed [0:2176) column range on
every core, so all 8 cores run one SPMD program; per-core differences
live in the data (rolled input, boundary-clamp masks).
"""

import sys

sys.path.insert(0, "/opt/trn_rl_repo")

import contextlib
import ctypes
import types

import numpy as np
import ml_dtypes

import concourse.bass as bass
import concourse.tile as tile
from concourse import mybir
from concourse.masks import make_identity

# ---------------------------------------------------------------- infra shims
# 1) walrus in this container rejects InstDrain with >2 sync waits; re-emit
#    the tile exit drain's waits as standalone SP wait_ge instructions.


def _patched_drain_and_barrier(self, tick_clock, wait_clock):
    from concourse.vector_clock import ScopedClock

    nc = self.nc
    dummy = mybir.InstNoOp(name="I-drain-wait-probe", ins=[], outs=[])
    dummy.engine = mybir.EngineType.SP
    wait_clock.add_sem_waits(dummy, ScopedClock({None: tick_clock.global_clock}))
    si = dummy.sync_info
    assert self.sems is not None
    id2h = {h.num: h for h in self.sems.allocated().values()}
    if si is not None:
        for w in si.on_wait:
            assert w.wait_mode == "sem-ge-imm", w
            nc.sync.wait_ge(id2h[w.id], w.wait_value)
    nc.sync.drain()
    nc.all_engine_barrier()
    popped = nc._tile_sem_poison_stack.pop()
    assert popped is self._sem_poison
    nc.clear_and_free_semaphores(list(self.sems.allocated().values()))
    nc.all_engine_barrier()


tile.TileContext._drain_and_barrier = _patched_drain_and_barrier


def _split_excess_waits(nc, limit=1):
    """walrus here rejects instructions with more than ~2 sync waits; hoist
    excess waits onto standalone InstEventSemaphore instructions inserted
    just before the over-subscribed instruction on the same engine."""
    n_split = 0
    for f in nc.m.functions:
        for b in f.blocks:
            insts = list(b.instructions)
            out = []
            for inst in insts:
                si = inst.sync_info
                waits = list(si.on_wait) if si is not None else []
                if len(waits) > limit:
                    keep = waits[: limit - 1] if limit > 1 else []
                    hoist = waits[limit - 1 :] if limit > 1 else waits
                    # leave room: keep limit-1 on the instruction, then one
                    # hoisted event-sem per remaining wait
                    for w in hoist[:-1] if limit > 1 else hoist:
                        ev = mybir.InstEventSemaphore(
                            name=f"I-waitsplit-{nc.next_id()}", ins=[], outs=[]
                        )
                        ev.engine = inst.engine
                        ev.sync_info = mybir.SyncInfo(on_wait=[w], on_update=[])
                        nc.register_instruction(ev)
                        out.append(ev)
                        n_split += 1
                    if limit > 1:
                        keep = keep + [hoist[-1]]
                    inst.sync_info = mybir.SyncInfo(
                        on_wait=keep, on_update=list(si.on_update)
                    )
                out.append(inst)
            b.instructions = out
    return n_split

# 2) antenv.axon_hooks is missing in this image; provide it so
#    run_bass_kernel_spmd(trace=True) can capture NTFF profiles.


def _install_ntff_hook():
    def _make_hook():
        try:
            lib = ctypes.CDLL("/opt/axon/libaxon_pjrt.so")
        except OSError:
            return None
        if not hasattr(lib, "axon_start_nrt_profile"):
            return None
        lib.axon_start_nrt_profile.argtypes = [
            ctypes.POINTER(ctypes.c_int64),
            ctypes.c_size_t,
        ]
        lib.axon_start_nrt_profile.restype = ctypes.c_int64
        lib.axon_stop_nrt_profile.argtypes = [ctypes.c_char_p]
        lib.axon_stop_nrt_profile.restype = ctypes.c_int64

        @contextlib.contextmanager
        def _hook(output_dir, device_ids):
            import jax

            jax.devices()
            if device_ids:
                ids = (ctypes.c_int64 * len(device_ids))(*device_ids)
                rc = lib.axon_start_nrt_profile(ids, len(device_ids))
            else:
                rc = lib.axon_start_nrt_profile(None, 0)
            if rc != 0:
                raise RuntimeError(f"axon_start_nrt_profile rc={rc}")
            try:
                yield
            finally:
                n = lib.axon_stop_nrt_profile(str(output_dir).encode())
                print(f"ntff profile: {n} file(s) -> {output_dir}", file=sys.stderr)

        return _hook

    hook = _make_hook()
    mod = types.ModuleType("antenv.axon_hooks")
    mod.get_axon_ntff_profile_hook = lambda: hook
    mod.set_axon_ntff_profile_hook = lambda h: None
    sys.modules.setdefault("antenv.axon_hooks", mod)


_install_ntff_hook()

# ------------------------------------------------------------------ constants
B, C, H, W = 4, 64, 128, 128
Hh, Wh = H // 2, W // 2          # 64, 64
NQ = Hh * Wh                     # 4096 half-res pixels
C4 = 4 * C                       # 256
WIN = 34                         # query-window rows (32 out + 1 halo each side)
JW = WIN * Wh                    # 2176 query columns
JPASSES = [(0, 512), (512, 512), (1024, 512), (1536, 512), (2048, 128)]

F32 = mybir.dt.float32
BF16 = mybir.dt.bfloat16
FP8 = mybir.dt.float8e4
DR = mybir.MatmulPerfMode.DoubleRow
BF = ml_dtypes.bfloat16

EXP = mybir.ActivationFunctionType.Exp
SQRT = mybir.ActivationFunctionType.Sqrt
MULT = mybir.AluOpType.mult
ADD = mybir.AluOpType.add
SUB = mybir.AluOpType.subtract


# ------------------------------------------------------------- device program
def build_program():
    nc = bass.Bass("TRN2", target_bir_lowering=False, debug=False, num_devices=8)

    xmb_e = nc.dram_tensor("xmb", [128, 2, NQ], BF16, kind="ExternalInput").ap()
    xm2b_e = nc.dram_tensor("xm2b", [128, 2, NQ], BF16, kind="ExternalInput").ap()
    xres_e = nc.dram_tensor("xres", [64, Hh, W], F32, kind="ExternalInput").ap()
    lnw_e = nc.dram_tensor("lnw", [128, 2, 64], BF16, kind="ExternalInput").ap()
    lnst_e = nc.dram_tensor("lnst", [128, 4, 2], BF16, kind="ExternalInput").ap()
    gbt_e = nc.dram_tensor("gbt", [2, 64], BF16, kind="ExternalInput").ap()
    wq_e = nc.dram_tensor("wq", [65, 2, 32], BF16, kind="ExternalInput").ap()
    wk_e = nc.dram_tensor("wk", [65, 2, 32], BF16, kind="ExternalInput").ap()
    wv_e = nc.dram_tensor("wv", [65, 64], BF16, kind="ExternalInput").ap()
    wpix_e = nc.dram_tensor("wpix", [65, 192], BF16, kind="ExternalInput").ap()
    w1t_e = nc.dram_tensor("w1t", [128, 64], BF16, kind="ExternalInput").ap()
    bvec_e = nc.dram_tensor("bvec", [64, 10], F32, kind="ExternalInput").ap()
    out_e = nc.dram_tensor("out", [64, Hh, W], F32, kind="ExternalOutput").ap()

    with tile.TileContext(nc) as tc:
        with (
            tc.tile_pool(name="consts", bufs=1) as consts,
            tc.tile_pool(name="persist", bufs=1) as big,
            tc.tile_pool(name="norm", bufs=2) as normp,
            tc.tile_pool(name="dscr", bufs=1, space="DRAM") as dscr,
        ):
            # ---- constant loads
            lnw = consts.tile([128, 2, 64], BF16)
            nc.sync.dma_start(out=lnw, in_=lnw_e)
            lnst = consts.tile([128, 4, 2], BF16)
            nc.sync.dma_start(out=lnst, in_=lnst_e)
            gbt = consts.tile([2, 64], BF16)
            nc.sync.dma_start(out=gbt, in_=gbt_e)
            wq = consts.tile([65, 2, 32], BF16)
            nc.sync.dma_start(out=wq, in_=wq_e)
            wk = consts.tile([65, 2, 32], BF16)
            nc.sync.dma_start(out=wk, in_=wk_e)
            wv = consts.tile([65, 64], BF16)
            nc.sync.dma_start(out=wv, in_=wv_e)
            wpix = consts.tile([65, 192], BF16)
            nc.sync.dma_start(out=wpix, in_=wpix_e)
            w1t = consts.tile([128, 64], BF16)
            nc.sync.dma_start(out=w1t, in_=w1t_e)
            bvec = consts.tile([64, 10], F32)
            nc.sync.dma_start(out=bvec, in_=bvec_e)
            eps = consts.tile([128, 1], F32)
            nc.gpsimd.memset(eps, 1e-5)
            ident = consts.tile([64, 64], BF16)
            make_identity(nc, ident)

            # PE warm-up: ~6us of dummy matmuls so HAM unthrottles while the
            # input DMA is in flight
            wdum = consts.tile([128, 512], BF16)
            nc.gpsimd.memset(wdum, 0.25)
            with tc.tile_pool(name="psw", bufs=1, space="PSUM") as psw:
                ps_w = psw.tile([128, 512], F32)
                for _ in range(14):
                    nc.tensor.matmul(
                        ps_w, lhsT=wdum[:, 0:128], rhs=wdum,
                        start=True, stop=True,
                    )

            # ---- input + elementwise prep
            xres = big.tile([64, Hh, W], F32)
            nc.sync.dma_start(out=xres, in_=xres_e)
            c1 = big.tile([65, NQ], BF16)
            nc.gpsimd.memset(c1[64:65, :], 1.0)

            # ---- phase 1: patch-merge linear with folded LayerNorm.
            # Stats (col-sums of m, m^2) go to small psum tiles first; the
            # main linear runs afterwards with a K=2 fixup row pair carrying
            # the G*(-mu) and Bc*(1/rstd) terms, so no wide psum barrier.
            ps1 = tc.alloc_tile_pool(name="ps12", bufs=2, space="PSUM")
            ps2 = ps1  # shared psum pool across phases 1-2 (no bank barrier)
            ph1b = tc.alloc_tile_pool(name="ph1buf", bufs=1)
            if True:
                mbf = ph1b.tile([128, 2, NQ], BF16)
                m2bf = ph1b.tile([128, 2, NQ], BF16)
                for dc in range(4):
                    dsl = slice(dc * 1024, (dc + 1) * 1024)
                    nc.sync.dma_start(out=mbf[:, :, dsl], in_=xmb_e[:, :, dsl])
                    nc.sync.dma_start(out=m2bf[:, :, dsl], in_=xm2b_e[:, :, dsl])
                stat_sb = ph1b.tile([2, NQ], BF16, tag="statsb")
                for jt in range(8):
                    sl = slice(jt * 512, (jt + 1) * 512)
                    ps_st = ps1.tile([2, 512], F32, tag="st")
                    for ck in range(4):
                        rhs = (mbf if ck < 2 else m2bf)[:, ck % 2, sl]
                        nc.tensor.matmul(
                            ps_st,
                            lhsT=lnst[:, ck, :],
                            rhs=rhs,
                            start=(ck == 0),
                            stop=(ck == 3),
                        )
                    nc.vector.tensor_copy(out=stat_sb[:, sl], in_=ps_st)
                dstat = dscr.tile([2, NQ], BF16)
                nc.sync.dma_start(out=dstat, in_=stat_sb)
                st = consts.tile([128, 2, 32], BF16)
                nc.sync.dma_start(
                    out=st, in_=dstat[:, :].rearrange("k (p t) -> p k t", t=32)
                )
                mu = consts.tile([128, 32], F32)
                nc.vector.tensor_scalar_mul(out=mu, in0=st[:, 0, :], scalar1=1.0 / C4)
                var = consts.tile([128, 32], F32)
                nc.vector.tensor_mul(out=var, in0=mu, in1=mu)
                nc.vector.scalar_tensor_tensor(
                    out=var, in0=st[:, 1, :], scalar=1.0 / C4, in1=var,
                    op0=MULT, op1=SUB,
                )
                sa = consts.tile([128, 32], F32)  # sqrt(var+eps) = 1/rstd
                nc.scalar.activation(out=sa, in_=var, func=SQRT, bias=eps)
                ra = consts.tile([128, 32], F32)  # rstd
                nc.vector.reciprocal(out=ra, in_=sa)
                rows3 = consts.tile([128, 3, 32], BF16)
                nc.vector.tensor_scalar_mul(
                    out=rows3[:, 0, :], in0=mu, scalar1=-1.0
                )
                nc.vector.tensor_copy(out=rows3[:, 1, :], in_=sa)
                nc.vector.tensor_copy(out=rows3[:, 2, :], in_=ra)

                drow = dscr.tile([3, NQ], BF16)
                nc.sync.dma_start(
                    out=drow[:, :].rearrange("r (p t) -> p r t", t=32), in_=rows3
                )
                a_bc = ph1b.tile([64, NQ], BF16)
                nc.sync.dma_start(
                    out=a_bc, in_=drow[2:3, :].to_broadcast((64, NQ))
                )
                nsrow = ph1b.tile([2, NQ], BF16)
                nc.sync.dma_start(out=nsrow, in_=drow[0:2, :])

                # main linear + K=2 stats fixup, then scale by rstd
                for jt in range(8):
                    sl = slice(jt * 512, (jt + 1) * 512)
                    ps_c = ps1.tile([64, 512], F32, tag="c")
                    nc.tensor.matmul(
                        ps_c, lhsT=lnw[:, 0, :], rhs=mbf[:, 0, sl],
                        start=True, stop=False,
                    )
                    nc.tensor.matmul(
                        ps_c, lhsT=lnw[:, 1, :], rhs=mbf[:, 1, sl],
                        start=False, stop=False,
                    )
                    nc.tensor.matmul(
                        ps_c, lhsT=gbt, rhs=nsrow[:, sl],
                        start=False, stop=True,
                    )
                    nc.vector.tensor_mul(
                        out=c1[0:64, sl], in0=ps_c, in1=a_bc[:, sl]
                    )

            ph1b.release()

            # ---- phase 2: q/k/v, pixel-major combo, channel attention.
            # q/k land in fp8 [32, 2, n] DoubleRow layout (c = plane*32 + p);
            # biases are folded into the matmuls via c1's ones row.
            q = big.tile([32, 2, JW], FP8)
            k = big.tile([32, 2, NQ], FP8)
            v = big.tile([64, JW], BF16)
            pix = big.tile([128, 32, 3, 80], FP8)  # [vT|1], q2T, k2T per chunk
            nc.gpsimd.memset(pix[:, :, 0, 64:65], 1.0)
            cat = big.tile([128, JW], BF16)

            if True:
                for jt in range(8):
                    sl = slice(jt * 512, (jt + 1) * 512)
                    for hf in range(2):
                        ps_k = ps2.tile([32, 512], F32, tag="c")
                        nc.tensor.matmul(
                            ps_k, lhsT=wk[:, hf, :], rhs=c1[:, sl],
                            start=True, stop=True,
                        )
                        nc.vector.tensor_copy(out=k[:, hf, sl], in_=ps_k)
                for jt in range(5):
                    j0 = jt * 512
                    jw = min(512, JW - j0)
                    sl = slice(j0, j0 + jw)
                    for hf in range(2):
                        ps_q = ps2.tile([32, 512], F32, tag="c")
                        nc.tensor.matmul(
                            ps_q[:, 0:jw], lhsT=wq[:, hf, :], rhs=c1[:, sl],
                            start=True, stop=True,
                        )
                        nc.vector.tensor_copy(out=q[:, hf, sl], in_=ps_q[:, 0:jw])
                    ps_v = ps2.tile([64, 512], F32, tag="c")
                    nc.tensor.matmul(
                        ps_v[:, 0:jw], lhsT=wv, rhs=c1[:, sl], start=True, stop=True
                    )
                    nc.vector.tensor_copy(out=v[:, sl], in_=ps_v[:, 0:jw])
                # pixel-major [vT | q2T | k2T] (+b4,b5,b6) in one pass
                for it in range(32):
                    sl = slice(it * 128, (it + 1) * 128)
                    ps_px = ps2.tile([128, 192], F32, tag="px")
                    nc.tensor.matmul(
                        ps_px, lhsT=c1[:, sl], rhs=wpix, start=True, stop=True
                    )
                    nc.vector.tensor_copy(
                        out=pix[:, it, :, 0:64],
                        in_=ps_px[:, :].rearrange("p (a b) -> p a b", b=64),
                    )
                # channel attention
                ps_s2 = ps2.tile([64, 64], F32, tag="px")
                for it in range(32):
                    nc.tensor.matmul(
                        ps_s2,
                        lhsT=pix[:, it, 1, 0:64],
                        rhs=pix[:, it, 2, 0:64],
                        start=(it == 0),
                        stop=(it == 31),
                    )
                e2 = consts.tile([64, 64], F32)
                rs2 = consts.tile([64, 1], F32)
                nc.scalar.activation(out=e2, in_=ps_s2, func=EXP, accum_out=rs2)
                rr2 = consts.tile([64, 1], F32)
                nc.vector.reciprocal(out=rr2, in_=rs2)
                p2 = consts.tile([64, 64], BF16)
                nc.vector.tensor_scalar_mul(out=p2, in0=e2, scalar1=rr2)
                ps_t = ps2.tile([64, 64], BF16, tag="px")
                nc.tensor.transpose(ps_t, in_=p2, identity=ident)
                p2t = consts.tile([64, 64], BF16)
                nc.vector.tensor_copy(out=p2t, in_=ps_t)
                out2 = big.tile([64, JW], BF16)
                for jt in range(5):
                    j0 = jt * 512
                    jw = min(512, JW - j0)
                    ps_o2 = ps2.tile([64, 512], F32, tag="c")
                    nc.tensor.matmul(
                        ps_o2[:, 0:jw], lhsT=p2t, rhs=v[:, j0 : j0 + jw],
                        start=True, stop=True,
                    )
                    nc.vector.tensor_copy(
                        out=out2[:, j0 : j0 + jw], in_=ps_o2[:, 0:jw]
                    )
                nc.sync.dma_start(out=cat[64:128, :], in_=out2)
            ps1.release()

            # ---- phase 3: spatial attention (transposed scores, deferred
            #      softmax normalization via ones-row denominators) + conv1
            z = big.tile([64, WIN, Wh], BF16)
            z2 = z[:, :, :].rearrange("c h w -> c (h w)")
            dz = dscr.tile([1, JW], F32)
            with (
                tc.tile_pool(name="ps3", bufs=3, space="PSUM") as ps3,
                tc.tile_pool(name="psacc", bufs=2, space="PSUM") as psacc,
                tc.tile_pool(name="etp", bufs=3) as etp,
                tc.tile_pool(name="ph3n", bufs=2) as ph3n,
            ):
                for (j0, jw) in JPASSES:
                    ps_acc = psacc.tile([65, 512], F32, tag="acc")
                    # group i-chunks so each exp op covers ~1024 psum columns;
                    # grp is even so the aug matmul can consume chunk PAIRS
                    # via fp8 DoubleRow (contraction 2x128 per matmul)
                    grp = max(2, 1024 // jw)
                    for g0 in range(0, 32, grp):
                        ps_s = ps3.tile([128, 1024], F32, tag="s")
                        for c in range(grp):
                            it = g0 + c
                            isl = slice(it * 128, (it + 1) * 128)
                            nc.tensor.matmul(
                                ps_s[:, c * jw : c * jw + jw],
                                lhsT=k[:, :, isl],
                                rhs=q[:, :, j0 : j0 + jw],
                                start=True,
                                stop=True,
                                perf_mode=DR,
                            )
                        eT = etp.tile([128, 1024], FP8, tag="eT")
                        nw = grp * jw
                        nc.scalar.activation(
                            out=eT[:, 0:nw], in_=ps_s[:, 0:nw], func=EXP
                        )
                        for pr in range(grp // 2):
                            it = g0 + 2 * pr
                            nc.tensor.matmul(
                                ps_acc[:, 0:jw],
                                lhsT=pix[:, it : it + 2, 0, 0:65],
                                rhs=eT[
                                    :, pr * 2 * jw : (pr + 1) * 2 * jw
                                ].rearrange("p (two j) -> p two j", two=2),
                                start=(it == 0),
                                stop=(it == 30),
                                perf_mode=DR,
                            )
                    # normalize out1 by the ones-row denominator
                    rd = ph3n.tile([65, 512], F32, tag="rd")
                    nc.vector.reciprocal(
                        out=rd[64:65, 0:jw], in_=ps_acc[64:65, 0:jw]
                    )
                    nc.sync.dma_start(
                        out=dz[0:1, j0 : j0 + jw], in_=rd[64:65, 0:jw]
                    )
                    rb = ph3n.tile([64, 512], F32, tag="rb")
                    nc.sync.dma_start(
                        out=rb[:, 0:jw],
                        in_=dz[0:1, j0 : j0 + jw].to_broadcast((64, jw)),
                    )
                    nc.vector.tensor_mul(
                        out=cat[0:64, j0 : j0 + jw],
                        in0=ps_acc[0:64, 0:jw],
                        in1=rb[:, 0:jw],
                    )

            # conv1 over the whole window (after attention psum pools close)
            with tc.tile_pool(name="ps4", bufs=2, space="PSUM") as ps4:
                for h in range(0, JW, 512):
                    hw = min(512, JW - h)
                    ps_z = ps4.tile([64, 512], F32, tag="z")
                    nc.tensor.matmul(
                        ps_z[:, 0:hw],
                        lhsT=w1t,
                        rhs=cat[:, h : h + hw],
                        start=True,
                        stop=True,
                    )
                    nc.vector.tensor_scalar_add(
                        out=z2[:, h : h + hw],
                        in0=ps_z[:, 0:hw],
                        scalar1=bvec[:, 5:6],
                    )

            # ---- phase 4: bilinear upsample x2 + residual
            tailb = tc.alloc_tile_pool(name="tailbuf", bufs=1)
            dv = tailb.tile([64, 33, Wh], BF16)  # z'[t] - z'[t+1]
            nc.vector.tensor_sub(out=dv, in0=z[:, 0:33, :], in1=z[:, 1:34, :])
            upv = tailb.tile([64, Hh, Wh], BF16)
            upv_r = upv[:, :, :].rearrange("c (t two) w -> c t two w", two=2)
            nc.vector.scalar_tensor_tensor(
                out=upv_r[:, :, 0, :], in0=dv[:, 0:32, :], scalar=0.25,
                in1=z[:, 1:33, :], op0=MULT, op1=ADD,
            )
            nc.vector.scalar_tensor_tensor(
                out=upv_r[:, :, 1, :], in0=dv[:, 1:33, :], scalar=-0.25,
                in1=z[:, 1:33, :], op0=MULT, op1=ADD,
            )
            # boundary clamp corrections (maskA/maskB nonzero on edge cores)
            nc.vector.scalar_tensor_tensor(
                out=upv[:, 0, :], in0=dv[:, 0, :], scalar=bvec[:, 6:7],
                in1=upv[:, 0, :], op0=MULT, op1=ADD,
            )
            nc.vector.scalar_tensor_tensor(
                out=upv[:, Hh - 1, :], in0=dv[:, 32, :], scalar=bvec[:, 7:8],
                in1=upv[:, Hh - 1, :], op0=MULT, op1=ADD,
            )
            # horizontal
            dhh = tailb.tile([64, Hh, 63], BF16)
            nc.vector.tensor_sub(
                out=dhh, in0=upv[:, :, 0:63], in1=upv[:, :, 1:64]
            )
            uph = tailb.tile([64, Hh, W], BF16)
            uph_r = uph[:, :, :].rearrange("c h (s two) -> c h s two", two=2)
            nc.vector.scalar_tensor_tensor(
                out=uph_r[:, :, 1:64, 0], in0=dhh, scalar=0.25,
                in1=upv[:, :, 1:64], op0=MULT, op1=ADD,
            )
            nc.vector.scalar_tensor_tensor(
                out=uph_r[:, :, 0:63, 1], in0=dhh, scalar=-0.25,
                in1=upv[:, :, 0:63], op0=MULT, op1=ADD,
            )
            nc.vector.tensor_copy(out=uph_r[:, :, 0, 0], in_=upv[:, :, 0])
            nc.vector.tensor_copy(out=uph_r[:, :, 63, 1], in_=upv[:, :, 63])
            # residual add, f32 out
            outb = tailb.tile([64, Hh, W], F32)
            nc.vector.tensor_add(out=outb, in0=uph, in1=xres)
            nc.sync.dma_start(out=out_e, in_=outb)
            tailb.release()

    _split_excess_waits(nc)
    return nc


# ------------------------------------------------------------- host-side prep
def prepare_params(
    pm_gamma, pm_beta, pm_w, pm_b, w1, b1, w2, b2, w3, b3, w4, b4, w5, b5, w6, b6
):
    f = np.float32
    pm_gamma, pm_beta, pm_w, pm_b = (
        np.asarray(a, f) for a in (pm_gamma, pm_beta, pm_w, pm_b)
    )
    wg = pm_w * pm_gamma[None, :]           # [64, 256]
    G = wg.sum(1)                           # [64]
    Bc = pm_w @ pm_beta + pm_b              # [64]
    lnw = np.zeros((128, 2, 64), f)
    for ck in range(2):
        lnw[:, ck, :] = wg[:, ck * 128 : (ck + 1) * 128].T
    lnst = np.zeros((128, 4, 2), f)
    lnst[:, 0:2, 0] = 1.0
    lnst[:, 2:4, 1] = 1.0
    gbt = np.stack([G, Bc]).astype(f)        # [2, 64]
    wpix = np.zeros((65, 192), f)
    wpix[0:64, 0:64] = np.asarray(w4, f).T
    wpix[0:64, 64:128] = np.asarray(w5, f).T
    wpix[0:64, 128:192] = np.asarray(w6, f).T
    wpix[64, 0:64] = np.asarray(b4, f)
    wpix[64, 64:128] = np.asarray(b5, f)
    wpix[64, 128:192] = np.asarray(b6, f)

    def fold_qk(w, b):
        out = np.zeros((65, 2, 32), f)
        wt = np.asarray(w, f).T  # [c, o]
        for hf in range(2):
            out[0:64, hf, :] = wt[:, hf * 32 : (hf + 1) * 32]
            out[64, hf, :] = np.asarray(b, f)[hf * 32 : (hf + 1) * 32]
        return out

    wv_a = np.zeros((65, 64), f)
    wv_a[0:64] = np.asarray(w4, f).T
    wv_a[64] = np.asarray(b4, f)
    common = {
        "lnw": np.ascontiguousarray(lnw.astype(BF)),
        "lnst": np.ascontiguousarray(lnst.astype(BF)),
        "gbt": np.ascontiguousarray(gbt.astype(BF)),
        "wq": np.ascontiguousarray(fold_qk(w2, b2).astype(BF)),
        "wk": np.ascontiguousarray(fold_qk(w3, b3).astype(BF)),
        "wv": np.ascontiguousarray(wv_a.astype(BF)),
        "wpix": np.ascontiguousarray(wpix.astype(BF)),
        "w1t": np.ascontiguousarray(np.asarray(w1, f).T.astype(BF)),
    }
    bv = np.zeros((64, 10), f)
    for i, b in enumerate((b2, b3, b4, b5, b6, b1)):
        bv[:, i] = np.asarray(b, f)
    bv[:, 8] = G
    bv[:, 9] = Bc
    return common, bv


def make_xm(xb):
    """rolled x[b] [64, 128, 128] -> quadrant layout [128, 2, 4096]."""
    m = np.concatenate(
        [xb[:, 0::2, 0::2], xb[:, 1::2, 0::2], xb[:, 0::2, 1::2], xb[:, 1::2, 1::2]],
        axis=0,
    ).reshape(C4, NQ)
    return np.ascontiguousarray(m.reshape(2, 128, NQ).transpose(1, 0, 2))


def make_in_maps(inputs):
    x = np.asarray(inputs["x"], np.float32)
    common, bv = prepare_params(**{kk: vv for kk, vv in inputs.items() if kk != "x"})
    in_maps = []
    for core in range(8):
        b, half = core // 2, core % 2
        shift = 2 - 64 * half  # rolled[rf] = real[rf - shift]
        xr = np.roll(x[b], shift, axis=1)
        bvc = bv.copy()
        bvc[:, 6] = -0.25 if half == 0 else 0.0
        bvc[:, 7] = 0.25 if half == 1 else 0.0
        xres = np.ascontiguousarray(x[b][:, 64 * half : 64 * half + 64, :])
        m = make_xm(xr)
        in_maps.append(
            dict(
                common,
                xmb=np.ascontiguousarray(m.astype(BF)),
                xm2b=np.ascontiguousarray((m * m).astype(BF)),
                xres=xres,
                bvec=bvc,
            )
        )
    return in_maps


def gather(results):
    out = np.zeros((B, C, H, W), np.float32)
    for core in range(8):
        b, half = core // 2, core % 2
        out[b, :, 64 * half : 64 * half + 64, :] = results[core]["out"]
    return out


_NC = None


def _get_nc():
    global _NC
    if _NC is None:
        _NC = build_program()
    return _NC


def run(inputs, trace=False, tmpdir=None):
    from concourse.bass_utils import run_bass_kernel_spmd

    res = run_bass_kernel_spmd(
        _get_nc(),
        make_in_maps(inputs),
        core_ids=list(range(8)),
        trace=trace,
        tmpdir=tmpdir,
    )
    return gather(res.results), res


def kernel(**inputs):
    return run(inputs)[0]
